# revision 10
# baseline (speedup 1.0000x reference)
"""Trainium2 Bass kernel for nn_MatchSegmentation.

Computes matching = argmin_g BCE(segmentation_k, gt_g) for K=128 proposals vs
G=gt_plane_num ground-truth masks over N=65536 pixels, sharded over the pixel
dimension across 8 NeuronCores.

Math: ce[k,g] = -(A[k,g] + B[k] - C[k,g]) / n with
  A = log(s+eps)   @ g^T,  C = log(1-s+eps) @ g^T,  B = rowsum(log(1-s+eps)).
B is constant per row and -1/n is a monotone-decreasing scale, so
  argmin_g ce[k,:] == argmax_g (A - C)[k,:].
Each core computes a partial (G, K) A|C pair over its pixel shard on the PE
(bf16 weights = gt masks, bf16 moving operand = concatenated logs), AllGathers
the per-core partials, sums + masks + transposes + arg-maxes on every core
(replicated), and core 0's output is returned.
"""

import numpy as np
import ml_dtypes
from contextlib import ExitStack

import concourse.bass as bass
import concourse.tile as tile
from concourse import bacc, mybir
from concourse.bass_utils import run_bass_kernel_spmd

F32 = mybir.dt.float32
BF16 = mybir.dt.bfloat16

NCORES = 8
N_FULL = 65536          # h*w pixels
K = 128                 # segmentation channels
GMAX = 21               # gt instances provided
GP = 22                 # padded instance slots (col 21 always padding)
NSHARD = N_FULL // NCORES   # 8192 pixels per core
CHUNK = 128             # pixels per matmul (contraction = partition dim)
NCHUNK = NSHARD // CHUNK    # 64
BLK_CH = 16             # chunks per pipeline block
NBLK = NCHUNK // BLK_CH     # 4
EPS = 1e-6

_PROG = None  # (nc, ) cached compiled program


def _build_program():
    nc = bacc.Bacc(
        "TRN2",
        target_bir_lowering=False,
        debug=False,
        enable_asserts=False,
        num_devices=NCORES,
    )

    seg_d = nc.dram_tensor("seg", [NSHARD, K], F32, kind="ExternalInput")
    gt_d = nc.dram_tensor("gt", [128, NCHUNK * GP], BF16, kind="ExternalInput")
    pen_d = nc.dram_tensor("pen", [GP, 1], F32, kind="ExternalInput")
    idn_d = nc.dram_tensor("idn", [GP, GP], F32, kind="ExternalInput")
    bias_d = nc.dram_tensor("bias2", [128, 2], F32, kind="ExternalInput")
    out_d = nc.dram_tensor("out", [K, 1], mybir.dt.int32, kind="ExternalOutput")

    with tile.TileContext(nc) as tc, ExitStack() as ctx:
        segp = ctx.enter_context(tc.tile_pool(name="segp", bufs=3))
        logp = ctx.enter_context(tc.tile_pool(name="logp", bufs=2))
        gtp = ctx.enter_context(tc.tile_pool(name="gtp", bufs=2))
        psp = ctx.enter_context(tc.tile_pool(name="psp", bufs=1, space="PSUM"))
        sml = ctx.enter_context(tc.tile_pool(name="sml", bufs=1))
        drm = ctx.enter_context(tc.tile_pool(name="drm", bufs=1, space="DRAM"))

        # Warm the ACT Log table while the first DMA is in flight.
        dummy = sml.tile([1, 8], F32)
        nc.vector.memset(dummy[:], 1.0)
        nc.scalar.activation(dummy[:], dummy[:], mybir.ActivationFunctionType.Ln)

        # Small constants.
        pen_t = sml.tile([GP, 1], F32)
        nc.sync.dma_start(pen_t[:], pen_d.ap())
        idn_t = sml.tile([GP, GP], F32)
        nc.sync.dma_start(idn_t[:], idn_d.ap())
        bias_t = sml.tile([128, 2], F32)
        nc.sync.dma_start(bias_t[:], bias_d.ap())

        # A|C accumulator: [:, :K] accumulates g^T@log_s, [:, K:] g^T@log_1ms.
        psAC = psp.tile([GP, 2 * K], F32)

        seg_v = seg_d.ap().rearrange("(b c p) k -> b p c k", c=BLK_CH, p=CHUNK)
        gt_v = gt_d.ap().rearrange("p (b f) -> b p f", b=NBLK)

        for b in range(NBLK):
            seg_t = segp.tile([128, BLK_CH, K], F32, name="seg_t")
            nc.sync.dma_start(seg_t[:], seg_v[b])
            gt_t = gtp.tile([128, BLK_CH, GP], BF16, name="gt_t")
            nc.sync.dma_start(gt_t[:], gt_v[b].rearrange("p (c j) -> p c j", c=BLK_CH))

            logs_t = logp.tile([128, BLK_CH, 2 * K], BF16, name="logs_t")
            # log(s + eps)
            nc.scalar.activation(
                logs_t[:, :, 0:K], seg_t[:],
                mybir.ActivationFunctionType.Ln, bias=bias_t[:, 0:1], scale=1.0,
            )
            # log(1 - s + eps) = log(-s + (1+eps))
            nc.scalar.activation(
                logs_t[:, :, K : 2 * K], seg_t[:],
                mybir.ActivationFunctionType.Ln, bias=bias_t[:, 1:2], scale=-1.0,
            )

            for c in range(BLK_CH):
                gc = b * BLK_CH + c
                nc.tensor.matmul(
                    psAC[:],
                    lhsT=gt_t[:, c, :],
                    rhs=logs_t[:, c, :],
                    start=(gc == 0),
                    stop=(gc == NCHUNK - 1),
                )

        # D_local = A - C  (GP, K)
        ac_sb = sml.tile([GP, 2 * K], F32)
        nc.vector.tensor_copy(ac_sb[:], psAC[:])
        dt_sb = sml.tile([GP, K], F32)
        nc.vector.tensor_sub(dt_sb[:], ac_sb[:, 0:K], ac_sb[:, K : 2 * K])

        # AllGather partials across the 8 cores, then reduce locally.
        cc_in = drm.tile([GP, K], F32)
        nc.sync.dma_start(cc_in[:], dt_sb[:])
        cc_out = drm.tile([NCORES * GP, K], F32, addr_space="Shared")
        nc.gpsimd.collective_compute(
            "AllGather",
            mybir.AluOpType.bypass,
            replica_groups=[list(range(NCORES))],
            ins=[cc_in.opt()],
            outs=[cc_out.opt()],
        )
        allg = sml.tile([GP, NCORES, K], F32)
        nc.sync.dma_start(allg[:], cc_out.rearrange("(r g) k -> g r k", r=NCORES))

        dt_sum = sml.tile([GP, K], F32)
        nc.vector.tensor_add(dt_sum[:], allg[:, 0, :], allg[:, 1, :])
        for r in range(2, NCORES):
            nc.vector.tensor_add(dt_sum[:], dt_sum[:], allg[:, r, :])

        # Mask out padded / out-of-range instance slots, then put K on partitions.
        nc.vector.tensor_scalar_add(dt_sum[:], dt_sum[:], pen_t[:])
        ps_t = psp.tile([K, GP], F32)
        nc.tensor.transpose(ps_t[:], dt_sum[:], idn_t[:])
        ce_t = sml.tile([K, GP], F32)
        nc.vector.tensor_copy(ce_t[:], ps_t[:])

        # argmax along the G axis == argmin of the BCE.
        mx = sml.tile([K, 8], F32)
        nc.vector.max(mx[:], ce_t[:])
        idx = sml.tile([K, 8], mybir.dt.uint32)
        nc.vector.max_index(idx[:], mx[:], ce_t[:])
        nc.sync.dma_start(out_d.ap(), idx[:, 0:1].bitcast(mybir.dt.int32))

    nc.compile()
    return nc


def _prepare_in_maps(segmentation, gt_instance, gt_plane_num):
    seg = np.ascontiguousarray(np.asarray(segmentation, dtype=np.float32))
    assert seg.shape == (N_FULL, K)
    gt = np.asarray(gt_instance)
    gmax = gt.shape[0]
    gpn = int(gt_plane_num)

    # (N, GP) bf16 mask matrix, padded columns zero.
    gpad = np.zeros((N_FULL, GP), dtype=np.float32)
    gpad[:, :gmax] = gt.reshape(gmax, -1).T
    gpad = gpad.astype(ml_dtypes.bfloat16)

    pen = np.zeros((GP, 1), dtype=np.float32)
    pen[min(gpn, GP):] = -1.0e30

    idn = np.eye(GP, dtype=np.float32)

    bias2 = np.empty((128, 2), dtype=np.float32)
    bias2[:, 0] = EPS
    bias2[:, 1] = 1.0 + EPS

    in_maps = []
    for c in range(NCORES):
        lo = c * NSHARD
        gt_core = (
            gpad[lo : lo + NSHARD]
            .reshape(NCHUNK, CHUNK, GP)
            .transpose(1, 0, 2)
            .reshape(CHUNK, NCHUNK * GP)
        )
        in_maps.append(
            {
                "seg": seg[lo : lo + NSHARD],
                "gt": np.ascontiguousarray(gt_core),
                "pen": pen,
                "idn": idn,
                "bias2": bias2,
            }
        )
    return in_maps


LAST_RESULTS = None


def run(inputs, trace=False, **kwargs):
    global _PROG, LAST_RESULTS
    if _PROG is None:
        _PROG = _build_program()
    in_maps = _prepare_in_maps(
        inputs["segmentation"], inputs["gt_instance"], inputs["gt_plane_num"]
    )
    res = run_bass_kernel_spmd(
        _PROG, in_maps, core_ids=list(range(NCORES)), trace=trace, **kwargs
    )
    LAST_RESULTS = res
    return np.asarray(res.results[0]["out"], dtype=np.int32)


def kernel(**inputs):
    return run(inputs)
